# revision 7
# baseline (speedup 1.0000x reference)
"""Bass/Trainium2 kernel for nn_DiagonalDense: y = x * diag_elements (column scaling).

Full input x is (16384, 2048) f32, diag_elements is (2048,) f32. Data-parallel
over 8 NeuronCores: each core handles a 2048-row shard; diag is replicated.
Per core: 16 MiB in + 16 MiB out of HBM traffic -> memory-bound, roofline
~94 us at ~358 GB/s HBM-per-core.
"""

import os

import numpy as np

import concourse.bacc as bacc
import concourse.bass as bass
import concourse.mybir as mybir
import concourse.tile as tile
from concourse.bass_utils import run_bass_kernel_spmd

N_CORES = 8
ROWS, COLS = 16384, 2048
SHARD_ROWS = ROWS // N_CORES  # 2048
P = 128
BLOCKS = SHARD_ROWS // P  # 16 row-blocks of 128 rows per shard

# Tunables: B row-blocks packed into one SBUF supertile [128, B*COLS].
B = int(os.environ.get("KERNEL_B", "2"))
BUFS = int(os.environ.get("KERNEL_BUFS", "4"))
N_SUPER = BLOCKS // B

_PROGRAM_CACHE: dict = {}


def _build_program() -> bass.Bass:
    nc = bacc.Bacc("TRN2")
    x = nc.dram_tensor("x", [SHARD_ROWS, COLS], mybir.dt.float32, kind="ExternalInput")
    d = nc.dram_tensor("d", [COLS], mybir.dt.float32, kind="ExternalInput")
    y = nc.dram_tensor("y", [SHARD_ROWS, COLS], mybir.dt.float32, kind="ExternalOutput")

    # Supertile n covers rows [n*B*P, (n+1)*B*P): partition p holds rows
    # n*B*P + b*P + p for b in [0, B), laid out as free index b*COLS + m.
    x_t = x.ap().rearrange("(n b p) m -> n b p m", p=P, b=B)
    y_t = y.ap().rearrange("(n b p) m -> n b p m", p=P, b=B)

    with tile.TileContext(nc) as tc:
        with (
            tc.tile_pool(name="const", bufs=1) as const_pool,
            tc.tile_pool(name="work", bufs=BUFS) as work_pool,
        ):
            diag = const_pool.tile([P, COLS], mybir.dt.float32)
            scratch = const_pool.tile([P, 1], mybir.dt.float32)
            # Broadcast-read the 8 KB diag vector into all 128 partitions.
            nc.gpsimd.dma_start(diag[:], d.ap().partition_broadcast(P))
            # Joiner: advance the vector engine's clock past the diag load
            # once, so the per-tile muls don't each carry a diag sync-wait
            # (the TT struct has a small sync-wait slot budget).
            nc.vector.tensor_copy(scratch[:], diag[:, 0:1])

            for n in range(N_SUPER):
                t = work_pool.tile([P, B * COLS], mybir.dt.float32)
                src = x_t[n].transpose([1, 0, 2])  # [P, B, COLS] view of DRAM
                dst = y_t[n].transpose([1, 0, 2])
                t3 = t[:].rearrange("p (b m) -> p b m", b=B)
                nc.sync.dma_start(t3, src)
                for b in range(B):
                    seg = t[:, b * COLS : (b + 1) * COLS]
                    nc.vector.tensor_mul(seg, seg, diag[:])
                nc.sync.dma_start(dst, t3)
    nc.compile()
    return nc


def _get_program() -> bass.Bass:
    key = (B, BUFS)
    if key not in _PROGRAM_CACHE:
        _PROGRAM_CACHE[key] = _build_program()
    return _PROGRAM_CACHE[key]


LAST_RESULT = None  # BassKernelResults of the most recent run (for profiling)


def kernel(x: np.ndarray, diag_elements: np.ndarray) -> np.ndarray:
    global LAST_RESULT
    x = np.ascontiguousarray(np.asarray(x), dtype=np.float32)
    d = np.ascontiguousarray(np.asarray(diag_elements), dtype=np.float32)
    assert x.shape == (ROWS, COLS) and d.shape == (COLS,)

    nc = _get_program()
    shards = x.reshape(N_CORES, SHARD_ROWS, COLS)
    in_maps = [{"x": shards[i], "d": d} for i in range(N_CORES)]
    trace = os.environ.get("KERNEL_PROFILE") == "1"
    LAST_RESULT = run_bass_kernel_spmd(
        nc, in_maps, list(range(N_CORES)), trace=trace
    )
    out = np.stack([r["y"] for r in LAST_RESULT.results], axis=0)
    return out.reshape(ROWS, COLS)
